# revision 1
# baseline (speedup 1.0000x reference)
"""DeformConv3D (3x3x3, pad 1, stride 1) on 8 Trainium2 NeuronCores — v2.

Sharding: core = b*4 + dq handles batch b, output d-planes [2*dq, 2*dq+2).

v2 changes vs baseline:
  - x transferred once as channels-last fp16 F [NPOS+4, 64]; the dual-parity
    w-pair union is built on device with two DRAM->DRAM DMA copies.
  - gather indices transferred compact [16, .] and replicated to 128
    partitions on device by doubling DMAs (8x less transfer).
  - trilinear corner weights expanded to full [128, jh, 2, 64] tiles on the
    Activation engine (mostly) so the DVE multiply runs packed at the 2x
    16-bit rate; remainder of expansions on DVE to balance engines.
  - PE transposes write 4 j-columns per PSUM bank; single batched
    PSUM->SBUF copy.
  - output stored fp16, converted to fp32 on host.
"""
import os
import numpy as np
from contextlib import ExitStack

import concourse.bacc as bacc
import concourse.mybir as mybir
import concourse.tile as tile
from concourse import library_config
from concourse.masks import make_identity
from concourse.bass_utils import run_bass_kernel_spmd

F16, F32, I16 = mybir.dt.float16, mybir.dt.float32, mybir.dt.int16
_NQUEUES = 4

B, C, D, H, W = 2, 64, 8, 56, 56
Cout, K = 64, 27
N_CORES = 8
DQ = 4
DO_SLAB = D // DQ              # 2
P_CORE = DO_SLAB * H * W       # 6272
NPOS = D * H * W               # 25088
NPAIR = NPOS // 2 + 1          # 12545
NU = 2 * NPAIR                 # 25090
JH = [25, 24]
HALF_N = [25 * 128, 24 * 128]
HALF_OFF = [0, 25 * 128]
NKP = 14
NF = NPOS + 4                  # F rows (one zero pad row in front, 3 after)
# fraction of (k,m) weight expansions done on DVE (rest on Act) for balance
_DVE_EXP_FRAC = float(os.environ.get("DEFORM_DVE_EXP", "0.0"))


def _chunks_of(n):
    out, c0 = [], 0
    while c0 < n:
        cs = min(512, n - c0)
        out.append((c0, cs))
        c0 += cs
    return out


def _build_kernel(nc, out, fsrc, union, idxA, idxB, wtsA, wtsB, wmat):
    nc.gpsimd.load_library(library_config.mlp)
    with tile.TileContext(nc) as tc, ExitStack() as ctx:
        const = ctx.enter_context(tc.tile_pool(name="const", bufs=1))
        idxp = ctx.enter_context(tc.tile_pool(name="idxp", bufs=4))
        wtp = ctx.enter_context(tc.tile_pool(name="wtp", bufs=3))
        wxp = ctx.enter_context(tc.tile_pool(name="wxp", bufs=3))
        gpool = ctx.enter_context(tc.tile_pool(name="gpool", bufs=3))
        tprod = ctx.enter_context(tc.tile_pool(name="tprod", bufs=3))
        colsp = ctx.enter_context(tc.tile_pool(name="colsp", bufs=3))
        tmpp = ctx.enter_context(tc.tile_pool(name="tmpp", bufs=2))
        rhsp = ctx.enter_context(tc.tile_pool(name="rhsp", bufs=1))
        outp = ctx.enter_context(tc.tile_pool(name="outp", bufs=3))
        psT = ctx.enter_context(tc.tile_pool(name="psT", bufs=4, space="PSUM"))
        psG = ctx.enter_context(tc.tile_pool(name="psG", bufs=2, space="PSUM"))

        # --- build the dual-parity union in DRAM from the single F copy ---
        fflat = fsrc.rearrange("r c -> (r c)")
        nc.sync.dma_start(
            union[0:NPAIR, :],
            fflat[0:NPAIR * 128].rearrange("(r c) -> r c", c=128))
        nc.scalar.dma_start(
            union[NPAIR:NU, :],
            fflat[64:64 + NPAIR * 128].rearrange("(r c) -> r c", c=128))

        ident = const.tile([128, 128], F16)
        make_identity(nc, ident[:])
        wm = const.tile([128, NKP, 64], F16)
        for kp in range(NKP):
            nc.sync.dma_start(wm[:, kp, :], wmat[kp])

        exp_i = 0  # rotating counter for DVE/Act expansion split
        n_dve_exp = int(round(_DVE_EXP_FRAC * K * 4 * 2))
        exp_total = K * 4 * 2

        for half in range(2):
            jh = JH[half]
            n = HALF_N[half]
            off = HALF_OFF[half]
            ncols = n // 16
            idx_dram = idxA if half == 0 else idxB
            wts_dram = wtsA if half == 0 else wtsB

            rhs = rhsp.tile([128, NKP, HALF_N[0]], F16, tag="rhs")
            # k=26 leaves rhs[64:, 13] unwritten; zero it so the 0-weight
            # matmul rows can't pull NaNs out of stale SBUF.
            nc.vector.memset(rhs[64:128, NKP - 1, :n], 0.0)

            for k in range(K):
                wt_t = wtp.tile([128, 8 * JH[0]], F16, tag="wt")
                nc.sync.dma_start(wt_t[:, :8 * jh], wts_dram[k])

                # compact idx [16, 4*ncols] -> replicate to 128 partitions
                idx_t = idxp.tile([128, 4 * (HALF_N[0] // 16)], I16, tag="idx")
                nc.sync.dma_start(idx_t[0:16, :4 * ncols], idx_dram[k])
                nc.sync.dma_start(idx_t[16:32, :4 * ncols],
                                  idx_t[0:16, :4 * ncols])
                nc.sync.dma_start(idx_t[32:64, :4 * ncols],
                                  idx_t[0:32, :4 * ncols])
                nc.sync.dma_start(idx_t[64:128, :4 * ncols],
                                  idx_t[0:64, :4 * ncols])

                colsE = colsp.tile([128, jh, 64], F16, tag="colsE")
                colsO = colsp.tile([128, jh, 64], F16, tag="colsO")
                for m in range(4):
                    g = gpool.tile([128, jh, 128], F16, tag="g")
                    nc.gpsimd.dma_gather(
                        g[:], union[:, :],
                        idx_t[:, m * ncols:(m + 1) * ncols], n, n, 128,
                        single_packet=False,
                        queue_num=(k * 4 + m) % _NQUEUES,
                    )
                    # expand weights [128, 2*jh] -> [128, jh, 2, 32]; the
                    # 32-wide tile is reused for both c-halves of the mult,
                    # halving expansion work vs a full 64-wide tile
                    wx = wxp.tile([128, JH[0], 2, 32], F16, tag="wx")
                    win = (wt_t[:, m * 2 * jh:(m + 1) * 2 * jh]
                           .rearrange("p (h j) -> p j h", h=2)
                           .to_broadcast([128, jh, 2, 32]))
                    use_dve = (exp_i * n_dve_exp) // exp_total != \
                              ((exp_i + 1) * n_dve_exp) // exp_total
                    exp_i += 1
                    if use_dve:
                        nc.vector.tensor_copy(out=wx[:, :jh], in_=win)
                    else:
                        nc.scalar.copy(out=wx[:, :jh], in_=win)
                    # packed 2x multiply: weighted pairs, c in 2 chunks
                    t = tprod.tile([128, jh, 128], F16, tag="t")
                    tv = t[:].rearrange("p j (h c) -> p j h c", h=2)
                    gv = g[:].rearrange("p j (h c) -> p j h c", h=2)
                    for cb in range(2):
                        nc.vector.tensor_tensor(
                            out=tv[:, :, :, cb * 32:(cb + 1) * 32],
                            in0=gv[:, :, :, cb * 32:(cb + 1) * 32],
                            in1=wx[:, :jh],
                            op=mybir.AluOpType.mult)
                    # pair combine; m0+m1 share colsE, m2+m3 share colsO;
                    # the E+O merge happens in PSUM via accumulating
                    # regular matmuls against the identity (== transpose)
                    acc = colsE if m < 2 else colsO
                    if m % 2 == 0:
                        nc.vector.tensor_tensor(
                            out=acc[:], in0=t[:, :, 0:64], in1=t[:, :, 64:128],
                            op=mybir.AluOpType.add)
                    else:
                        tp = tmpp.tile([128, jh, 64], F16, tag="tmp")
                        nc.vector.tensor_tensor(
                            out=tp[:], in0=t[:, :, 0:64], in1=t[:, :, 64:128],
                            op=mybir.AluOpType.add)
                        nc.vector.tensor_tensor(
                            out=acc[:], in0=acc[:], in1=tp[:],
                            op=mybir.AluOpType.add)

                kp, s = divmod(k, 2)
                # transpose cols to contraction-major; 8 j per PSUM bank,
                # one batched copy each
                for j0 in range(0, jh, 4):
                    jb = min(4, jh - j0)
                    pt = psT.tile([64, 4, 128], F32, tag="pt")
                    for j in range(jb):
                        nc.tensor.matmul(
                            out=pt[:, j, :], lhsT=colsE[:, j0 + j, :],
                            rhs=ident[:], start=True, stop=False)
                        nc.tensor.matmul(
                            out=pt[:, j, :], lhsT=colsO[:, j0 + j, :],
                            rhs=ident[:], start=False, stop=True)
                    nc.scalar.copy(
                        out=rhs[s * 64:(s + 1) * 64, kp,
                                j0 * 128:(j0 + jb) * 128],
                        in_=pt[:, :jb].rearrange("p j c -> p (j c)"))

            for (c0, cs) in _chunks_of(n):
                po = psG.tile([64, 512], F32, tag="po")
                for kp in range(NKP):
                    nc.tensor.matmul(
                        out=po[:, :cs], lhsT=wm[:, kp, :],
                        rhs=rhs[:, kp, c0:c0 + cs],
                        start=(kp == 0), stop=(kp == NKP - 1))
                ob = outp.tile([64, 512], F16, tag="ob")
                nc.vector.tensor_copy(out=ob[:, :cs], in_=po[:, :cs])
                nc.sync.dma_start(out[:, off + c0:off + c0 + cs], ob[:, :cs])


_NC_CACHE = None


def _get_nc():
    global _NC_CACHE
    if _NC_CACHE is None:
        nc = bacc.Bacc("TRN2", target_bir_lowering=False, debug=False,
                       num_devices=N_CORES, num_swdge_queues=_NQUEUES)
        fsrc = nc.dram_tensor("fsrc", [NF, C], F16, kind="ExternalInput")
        union = nc.dram_tensor("union", [NU, 2 * C], F16, kind="Internal")
        idxA = nc.dram_tensor("idxA", [K, 16, 4 * (HALF_N[0] // 16)], I16,
                              kind="ExternalInput")
        idxB = nc.dram_tensor("idxB", [K, 16, 4 * (HALF_N[1] // 16)], I16,
                              kind="ExternalInput")
        wtsA = nc.dram_tensor("wtsA", [K, 128, 8 * JH[0]], F16,
                              kind="ExternalInput")
        wtsB = nc.dram_tensor("wtsB", [K, 128, 8 * JH[1]], F16,
                              kind="ExternalInput")
        wmat = nc.dram_tensor("wmat", [NKP, 128, Cout], F16,
                              kind="ExternalInput")
        out = nc.dram_tensor("out", [Cout, P_CORE], F16, kind="ExternalOutput")
        _build_kernel(nc, out[:, :], fsrc, union, idxA, idxB, wtsA, wtsB,
                      wmat)
        nc.compile()
        _NC_CACHE = nc
    return _NC_CACHE


# ---------------- host-side prep ----------------

def _build_F(xb):
    x_cl = np.ascontiguousarray(np.asarray(xb).transpose(1, 2, 3, 0))
    x_cl = x_cl.reshape(NPOS, C)
    F = np.zeros((NF, C), np.float16)
    F[1:NPOS + 1] = x_cl.astype(np.float16)
    return F


def _host_idx_weights(off_core, dq):
    off = np.asarray(off_core).reshape(K, 3, P_CORE).astype(np.float32)
    pidx = np.arange(P_CORE)
    do = (pidx // (H * W)) + dq * DO_SLAB
    ho = (pidx // W) % H
    wo = pidx % W
    kk = np.arange(K)
    kd = (kk // 9).astype(np.float32)
    kh = ((kk // 3) % 3).astype(np.float32)
    kw = (kk % 3).astype(np.float32)

    pd = off[:, 0] + kd[:, None] + (do[None, :] - 1.0)
    ph = off[:, 1] + kh[:, None] + (ho[None, :] - 1.0)
    pw = off[:, 2] + kw[:, None] + (wo[None, :] - 1.0)

    d0 = np.floor(pd); fd = pd - d0
    h0 = np.floor(ph); fh = ph - h0
    w0 = np.floor(pw); fw = pw - w0

    w0c = np.clip(w0, -1, W - 1)
    vw0 = ((w0 >= 0) & (w0 <= W - 1)).astype(np.float32)
    vw1 = ((w0 >= -1) & (w0 <= W - 2)).astype(np.float32)
    ww0 = (1.0 - fw) * vw0
    ww1 = fw * vw1

    idx = np.zeros((K, 4, P_CORE), np.int16)
    wts = np.zeros((K, 4, 2, P_CORE), np.float16)
    for m, (bd, bh) in enumerate([(0, 0), (0, 1), (1, 0), (1, 1)]):
        dc = np.clip(d0 + bd, 0, D - 1)
        hc = np.clip(h0 + bh, 0, H - 1)
        vd = ((d0 + bd >= 0) & (d0 + bd <= D - 1)).astype(np.float32)
        vh = ((h0 + bh >= 0) & (h0 + bh <= H - 1)).astype(np.float32)
        wd = (fd if bd else 1.0 - fd) * vd
        wh = (fh if bh else 1.0 - fh) * vh
        lin = (dc * H + hc) * W + w0c
        i = lin + 1.0
        q = i % 2
        idx[:, m] = ((i - q) / 2 + q * NPAIR).astype(np.int16)
        wts[:, m, 0] = (wd * wh * ww0).astype(np.float16)
        wts[:, m, 1] = (wd * wh * ww1).astype(np.float16)
    return idx, wts


def _wrap_idx_compact(vals, n):
    """vals [..., n] -> wrapped [..., 16, n//16] (no replication)."""
    lead = vals.shape[:-1]
    w = vals.reshape(*lead, n // 16, 16)
    return np.ascontiguousarray(np.swapaxes(w, -1, -2))


def _pack_wmat(weight):
    wk = np.asarray(weight).reshape(Cout, C, K).astype(np.float16)
    lhsT = np.zeros((NKP, 128, Cout), np.float16)
    for kp in range(NKP):
        for s in range(2):
            k = 2 * kp + s
            if k < K:
                lhsT[kp, s * 64:(s + 1) * 64, :] = wk[:, :, k].T
    return lhsT


def _core_inputs(F_b, offset, lhsT, core):
    b, dq = core // DQ, core % DQ
    off_core = np.asarray(offset[b, :, dq * DO_SLAB:(dq + 1) * DO_SLAB])
    idx, wts = _host_idx_weights(off_core, dq)

    # compact wrapped idx: [K, m, 16, ncols] -> [K, 16, m*ncols]
    iA = _wrap_idx_compact(idx[:, :, :HALF_N[0]], HALF_N[0])
    idxA = np.ascontiguousarray(
        iA.transpose(0, 2, 1, 3).reshape(K, 16, 4 * (HALF_N[0] // 16)))
    iB = _wrap_idx_compact(idx[:, :, HALF_N[0]:], HALF_N[1])
    idxB = np.ascontiguousarray(
        iB.transpose(0, 2, 1, 3).reshape(K, 16, 4 * (HALF_N[1] // 16)))
    # weights: [K, 4, 2, P] -> per half [K, 128, (m,h,j)]
    wA = wts[:, :, :, :HALF_N[0]].reshape(K, 4, 2, JH[0], 128)
    wA = np.ascontiguousarray(
        wA.transpose(0, 4, 1, 2, 3).reshape(K, 128, 8 * JH[0]))
    wB = wts[:, :, :, HALF_N[0]:].reshape(K, 4, 2, JH[1], 128)
    wB = np.ascontiguousarray(
        wB.transpose(0, 4, 1, 2, 3).reshape(K, 128, 8 * JH[1]))
    return dict(fsrc=F_b, idxA=idxA, idxB=idxB, wtsA=wA, wtsB=wB,
                wmat=lhsT)


def make_in_maps(x, offset, weight):
    lhsT = _pack_wmat(weight)
    Fs = [_build_F(np.asarray(x)[b]) for b in range(B)]
    return [
        _core_inputs(Fs[core // DQ], np.asarray(offset), lhsT, core)
        for core in range(N_CORES)
    ]


def assemble_output(results):
    out = np.zeros((B, Cout, D, H, W), np.float32)
    for core in range(N_CORES):
        b, dq = core // DQ, core % DQ
        o = results[core]["out"].astype(np.float32)
        out[b, :, dq * DO_SLAB:(dq + 1) * DO_SLAB] = o.reshape(
            Cout, DO_SLAB, H, W)
    return out


def kernel(x, offset, weight):
    x = np.asarray(x)
    offset = np.asarray(offset)
    weight = np.asarray(weight)
    nc = _get_nc()
    in_maps = make_in_maps(x, offset, weight)
    res = run_bass_kernel_spmd(nc, in_maps, core_ids=list(range(N_CORES)))
    return assemble_output(res.results)

